# revision 9
# baseline (speedup 1.0000x reference)
"""Trainium2 Bass kernel for windowed mean-pooling (segment_reduce).

Computes, for each (batch b, window w):
    out[b, w, :] = mean over t in [begins[b,w], ends'[b,w]) of features[b, t, :]
where ends' = clip(ends, begins, begins + 8) (the reference gathers at most
MAX_WINDOW=8 tokens) and empty windows produce 0 (count clamped to >= 1).

Strategy (data-parallel over batch, one sample per NeuronCore):
  - Host splits features into bf16 hi + lo (F = hi + lo up to ~1e-5 rel),
    packed interleaved [T, 2, D] so one DMA descriptor set loads both.
    bf16 matmuls are 4x cheaper than fp32 on the PE (fp32 lowers to 2 HW
    passes); hi+lo recovers fp32-grade accuracy at half the fp32 PE cost.
  - Slab layout in SBUF: token t on partition (t % 128), K-tile (t // 128).
  - For each 128-window output block: out_block = S^T @ hi + S^T @ lo on
    the TensorEngine, where S[t, w] = (begins[w] <= t < ends[w]) is built
    on-chip by the VectorEngine from broadcast begins/ends rows using fused
    compare ops. Accumulate over the block's K-tiles in PSUM, scale rows by
    1/count on the ScalarEngine, DMA out.
  - Per-block K-tile ranges come from the host (actual index data), taking
    the union across the 8 cores so one SPMD program serves all cores
    (masks are zero outside a core's true range -> contributes nothing).
  - DMA engine assignment: feature slab on GPSIMD (SWDGE, keeps descriptor
    generation off the critical sequencers), metadata on SP, outputs on ACT.
"""

import os
import sys

import numpy as np

for _p in ("/opt/trn_rl_repo", "/root/.axon_site/_ro/trn_rl_repo"):
    if os.path.isdir(_p) and _p not in sys.path:
        sys.path.insert(0, _p)

from concourse import bacc, mybir  # noqa: E402
import concourse.tile as tile  # noqa: E402
from concourse.bass_utils import run_bass_kernel_spmd  # noqa: E402

B, T, D, W = 8, 4096, 768, 2048
MAXWIN = 8
P = 128
NBLK = W // P  # 16 window blocks of 128 windows
NKT = T // P  # 32 K-tiles of 128 tokens
FCH = 4  # K-tiles per feature-load DMA chunk
MCH = 512  # windows per metadata DMA chunk
F32 = mybir.dt.float32
BF16 = mybir.dt.bfloat16


def _build_program(klo, khi):
    """Build the SPMD Bass program given per-block K-tile ranges [klo, khi)."""
    nc = bacc.Bacc(None)

    feat = nc.declare_dram_parameter("fhl", [T, 2, D], BF16, isOutput=False)
    rows = nc.declare_dram_parameter("rows", [1, 2 * W], F32, isOutput=False)
    ioiv = nc.declare_dram_parameter("ioiv", [P, NKT + NBLK], F32, isOutput=False)
    out_d = nc.declare_dram_parameter("out", [W, D], F32, isOutput=True)

    # token t = n*128 + p  ->  [p, n, hl, d]; window w = i*128 + p -> [p, i, d]
    feat_r = feat[:].rearrange("(n p) h d -> p n h d", p=P)
    out_r = out_d[:].rearrange("(n p) d -> p n d", p=P)

    with tile.TileContext(nc) as tc:
        with (
            tc.tile_pool(name="metap", bufs=1) as meta_pool,
            tc.tile_pool(name="fslab", bufs=1) as f_pool,
            tc.tile_pool(name="m2p", bufs=4) as m2_pool,
            tc.tile_pool(name="maskp", bufs=8) as mask_pool,
            tc.tile_pool(name="outp", bufs=4) as out_pool,
            tc.tile_pool(name="psum", bufs=3, space="PSUM") as psum_pool,
            tc.tile_pool(name="psumb", bufs=2, space="PSUM") as psumb_pool,
        ):
            # begins/ends row vector + iota/inv-count metadata (tiny DMAs).
            rows_sb = meta_pool.tile([1, 2 * W], F32)
            nc.sync.dma_start(out=rows_sb[:], in_=rows[:])
            ioiv_sb = meta_pool.tile([P, NKT + NBLK], F32)
            nc.sync.dma_start(out=ioiv_sb[:], in_=ioiv[:])
            io_sb = ioiv_sb[:, 0:NKT]
            iv_sb = ioiv_sb[:, NKT : NKT + NBLK]

            # First feature chunks on the SP HWDGE ring (starts immediately);
            # the rest via GPSIMD SWDGE (Q7 needs ~5us to boot, then streams
            # without occupying the SP sequencer).
            f_tiles = []
            for j in range(NKT // FCH):
                ft = f_pool.tile([P, FCH, 2, D], BF16, name=f"fc{j}", tag=f"fc{j}")
                eng = nc.sync if j < 2 else nc.gpsimd
                eng.dma_start(out=ft[:], in_=feat_r[:, j * FCH : (j + 1) * FCH, :, :])
                f_tiles.append(ft)

            # Broadcast begins/ends rows across all 128 partitions with a
            # K=1 ones-matmul (fp32: index values up to 4096 stay exact),
            # then copy PSUM -> SBUF.  bb_sb[p, w] = begins[w] for all p.
            ones_sb = meta_pool.tile([1, P], F32)
            nc.vector.memset(ones_sb[:], 1.0)
            bb_sb = meta_pool.tile([P, W], F32)
            eb_sb = meta_pool.tile([P, W], F32)
            for c in range(2 * W // MCH):
                dst = bb_sb if c % 2 == 0 else eb_sb
                wlo = (c // 2) * MCH
                src = rows_sb[:, (c % 2) * W + wlo : (c % 2) * W + wlo + MCH]
                pb = psumb_pool.tile([P, MCH], F32, name=f"pb{c}", tag="pb")
                nc.tensor.matmul(pb[:], ones_sb[:], src, start=True, stop=True)
                nc.vector.tensor_copy(out=dst[:, wlo : wlo + MCH], in_=pb[:])

            # Selection masks, one [P, P] bf16 tile per (K-tile, block) pair,
            # in [token, window] layout: mask[p, w] = (b[w] <= t) * (e[w] > t)
            # with t = 128k + p. Emitted in k-major order so masks stream out
            # in roughly the order the PE consumes them.
            pairs = sorted(
                ((k, i) for i in range(NBLK) for k in range(klo[i], khi[i]))
            )
            masks = {}
            for k, i in pairs:
                bsl = bb_sb[:, i * P : (i + 1) * P]
                esl = eb_sb[:, i * P : (i + 1) * P]
                m2 = m2_pool.tile([P, P], F32, name=f"m2_{k}_{i}", tag="m2")
                msk = mask_pool.tile([P, P], BF16, name=f"mask_{k}_{i}", tag="mask")
                nc.vector.tensor_scalar(
                    m2[:], esl, io_sb[:, k : k + 1], None, mybir.AluOpType.is_gt
                )
                nc.vector.scalar_tensor_tensor(
                    msk[:], bsl, io_sb[:, k : k + 1], m2[:],
                    mybir.AluOpType.is_le, mybir.AluOpType.mult,
                )
                masks[(k, i)] = msk

            for i in range(NBLK):
                ps = psum_pool.tile([P, D], F32, name=f"ps{i}", tag="ps")
                for k in range(klo[i], khi[i]):
                    msk = masks[(k, i)]
                    first = k == klo[i]
                    last = k == khi[i] - 1
                    for h in range(2):  # hi, lo
                        rhs = f_tiles[k // FCH][:, k % FCH, h, :]
                        for n0, nn in ((0, 512), (512, 256)):
                            nc.tensor.matmul(
                                ps[:, n0 : n0 + nn], msk[:], rhs[:, n0 : n0 + nn],
                                start=(first and h == 0),
                                stop=(last and h == 1),
                            )
                os = out_pool.tile([P, D], F32, name=f"os{i}", tag="os")
                nc.scalar.mul(out=os[:], in_=ps[:], mul=iv_sb[:, i : i + 1])
                nc.scalar.dma_start(out=out_r[:, i, :], in_=os[:])

    nc.finalize()
    return nc


def _prepare(features, begins, ends):
    feats = np.asarray(features, dtype=np.float32)
    assert feats.shape == (B, T, D), feats.shape
    b = np.clip(np.asarray(begins).astype(np.int64), 0, T - 1)
    e = np.asarray(ends).astype(np.int64)
    # Reference gathers at most MAXWIN tokens starting at b; empty -> count 1.
    e_eff = np.clip(e, b, np.minimum(b + MAXWIN, T))
    counts = np.maximum(e_eff - b, 1).astype(np.float32)
    inv = (1.0 / counts).astype(np.float32)

    bw = b.reshape(B, NBLK, P)
    ew = e_eff.reshape(B, NBLK, P)
    klo_pc = bw.min(-1) // P  # [B, NBLK]
    khi_pc = (np.maximum(ew.max(-1) - 1, bw.min(-1)) // P) + 1
    klo = klo_pc.min(0).astype(int)
    khi = khi_pc.max(0).astype(int)
    khi = np.minimum(np.maximum(khi, klo + 1), NKT)

    # bf16 hi/lo split, interleaved [B, T, 2, D].
    import ml_dtypes

    hi = feats.astype(ml_dtypes.bfloat16)
    lo = (feats - hi.astype(np.float32)).astype(ml_dtypes.bfloat16)
    fhl = np.stack([hi, lo], axis=2)  # [B, T, 2, D]

    iota = (np.arange(NKT)[None, :] * P + np.arange(P)[:, None]).astype(np.float32)
    in_maps = []
    for c in range(B):
        rowsc = np.concatenate(
            [b[c].astype(np.float32), e_eff[c].astype(np.float32)]
        ).reshape(1, 2 * W)
        ioiv = np.ascontiguousarray(
            np.concatenate([iota, inv[c].reshape(NBLK, P).T], axis=1)
        )  # [P, NKT + NBLK]
        in_maps.append(
            {
                "fhl": np.ascontiguousarray(fhl[c]),
                "rows": np.ascontiguousarray(rowsc),
                "ioiv": ioiv,
            }
        )
    return list(klo), list(khi), in_maps


def run(features, begins, ends, trace=False):
    """Build + run on 8 NeuronCores; returns (output, BassKernelResults)."""
    klo, khi, in_maps = _prepare(features, begins, ends)
    nc = _build_program(klo, khi)
    res = run_bass_kernel_spmd(nc, in_maps, list(range(B)), trace=trace)
    out = np.stack([res.results[c]["out"] for c in range(B)], axis=0)
    return out, res


def kernel(features, begins, ends):
    out, _ = run(features, begins, ends, trace=False)
    return out
